# revision 79
# baseline (speedup 1.0000x reference)
"""Multi-head causal attention on 8 TRN2 NeuronCores.

Sharding: core c -> batch c//2, head-group c%2 (8 of 16 heads).
Wq/Wk/Wv column-sharded, Wo row-sharded; the Wo all-reduce is the host-side
sum of the two partial outputs per batch. Inputs cast to bf16 on host.

Per-core kernel (Bass/Tile), software-pipelined around the in-order PE
queue (later-emitted matmuls cannot fill earlier stalls, so independent
work is interleaved at emission time):
  per sq-tile t (512 queries), causality means attention over chunks
  0..4t+3 needs only K/V blocks <= t, so tile t+1's K/V/Q projection
  chains (kT = Wk^T x_k^T into [512, 2048]; v = x_v Wv with a 65th
  ones-column per head -> softmax denominator rides the PV matmul) are
  emitted as fillers at tile t's head-pair boundaries, keeping the PE
  dense (HAM stays at full clock) while ACT paces the attention exps.
  head pairs: attnT chunks [sk 128, sq 512] = kT^T qT (PE row groups
  0-63/64-127), exp on ACT, causal zeroing via gpsimd affine_select
  (identity-matmul additive mask for general masks), PV accumulates
  out^T[65, 512] (row 64 = denominator).
  epilogue (deferred one pair): 1/den = exp(-ln den) -- ln on ACT, one
  sel65 PE matmul broadcasts both heads' ln-dens, exp(scale=-1), then
  normalize-multiply and DMA into aoT. (A [1,512] DVE reciprocal is 8
  cyc/elem = 3.4us serial and stalled the PV pipeline.)
  Output projection of tile t-1 interleaved at pair boundaries.
Only one ACT table set (natural_log_exp_and_others) is allowed during
finalize, else the table chooser thrashes 1.3us reloads per epilogue.
"""

import os
import sys

for _p in ("/opt/trn_rl_repo", "/root/.axon_site/_ro/trn_rl_repo"):
    if os.path.isdir(_p) and _p not in sys.path:
        sys.path.insert(0, _p)

import numpy as np

import concourse.bass as bass  # noqa: F401
import concourse.tile as tile
from concourse import bacc, mybir
from concourse.bass_utils import run_bass_kernel_spmd

# This kernel's only ACT functions are Exp and Ln. The act-table-load
# inserter greedily picks the first set covering each function
# (exp_and_others for Exp, natural_log for Ln), thrashing a ~1.3us
# table reload on every softmax epilogue (33 loads/run). Restrict the
# chooser to the one set that holds both, keeping dict order so the
# emitted act_func_set_id still indexes act_info.json correctly.
_orig_get_act_tables = bacc.get_activation_tables


def _patched_get_act_tables(arch):
    t = _orig_get_act_tables(arch)
    keep = "natural_log_exp_and_others"
    if keep not in t:
        return t
    return {n: (fns if n == keep else set()) for n, fns in t.items()}

F32 = mybir.dt.float32
F32R = mybir.dt.float32r
BF16 = mybir.dt.bfloat16

B, S, D = 4, 2048, 1024
H, DH = 16, 64
SCALE = DH**-0.5
NCORES = 8
NHPC = 8
HDPC = NHPC * DH  # 512
SQT = 512
NSQT = S // SQT  # 4
SKC = 128
NSKC = S // SKC  # 16
NDC = D // 128  # 8
NMC = HDPC // 128  # 4
NEG = -1.0e30

CFG = {
    "in": "bf16",
    "qk": "bf16",
    "pv": "bf16",
    "ao": "bf16",
    "xbufs": 6,
    "qtbufs": 2,
    "ebufs": 5,
    "sbufs": 2,
    "psq": 3,
    "psv": 4,
    "pso": 1,
}

LAST_RESULTS = None

_DT = {"f32r": F32R, "bf16": BF16, "fp16": mybir.dt.float16}


def _mask_layout(mask: np.ndarray):
    """Blocks of [sk=128, sq=512].  Returns chunks[t] = list of
    (c, kind, arg): kind 'clear' (no masking), 'affine' (causal-style
    triangle, arg = affine base), or 'madd' (arg = packed additive tile idx).
    Fully-masked blocks are dropped."""
    chunks = []
    uniq = {}
    madds = []
    rr = np.arange(SKC)[:, None]
    jj = np.arange(SQT)[None, :]
    for t in range(NSQT):
        lst = []
        for c in range(NSKC):
            sub = mask[t * SQT : (t + 1) * SQT, c * SKC : (c + 1) * SKC]
            if sub.all():
                continue
            if not sub.any():
                lst.append((c, "clear", 0))
                continue
            subT = sub.T
            base = c * SKC - t * SQT
            if np.array_equal(subT, (rr + base) > jj):
                lst.append((c, "affine", base))
                continue
            key = subT.tobytes()
            if key not in uniq:
                madds.append(np.where(subT, NEG, 0.0).astype(np.float32))
                uniq[key] = len(madds) - 1
            lst.append((c, "madd", uniq[key]))
        assert lst, f"sq tile {t} fully masked"
        chunks.append(lst)
    madd_arr = (
        np.stack(madds) if madds else np.zeros((1, SKC, SQT), dtype=np.float32)
    )
    return chunks, madd_arr, bool(madds)


def _build_program(chunks, n_madd, use_madd, cfg, tick=False, reps=1):
    qk_dt = _DT[cfg["qk"]]
    pv_dt = _DT[cfg["pv"]]
    ao_dt = _DT[cfg["ao"]]

    nc = bacc.Bacc(
        "TRN2", target_bir_lowering=False, debug=False, num_devices=NCORES
    )
    if tick:
        tick_ap = nc.dram_tensor("tick", [128, 8], F32, kind="ExternalInput").ap()
    in_dt = _DT.get(cfg.get("in", "f32r"), BF16)
    xqT = nc.dram_tensor("xqT", [D, S], in_dt, kind="ExternalInput").ap()
    xkT = nc.dram_tensor("xkT", [D, S], in_dt, kind="ExternalInput").ap()
    xvT = nc.dram_tensor("xvT", [D, S], in_dt, kind="ExternalInput").ap()
    wq = nc.dram_tensor("wq", [D, HDPC], in_dt, kind="ExternalInput").ap()
    wk = nc.dram_tensor("wk", [D, HDPC], in_dt, kind="ExternalInput").ap()
    wv = nc.dram_tensor("wv", [D, HDPC], in_dt, kind="ExternalInput").ap()
    wo = nc.dram_tensor("wo", [HDPC, D], in_dt, kind="ExternalInput").ap()
    lgz = nc.dram_tensor("lgz", [65, 2, SQT], F32, kind="ExternalInput").ap()
    if use_madd:
        madd = nc.dram_tensor(
            "madd", [n_madd, SKC, SQT], F32, kind="ExternalInput"
        ).ap()
    out = nc.dram_tensor("out", [S, D], F32, kind="ExternalOutput").ap()

    with tile.TileContext(nc) as tc:
        with (
            tc.tile_pool(name="const", bufs=1) as const,
            tc.tile_pool(name="wbig", bufs=3) as wbig,
            tc.tile_pool(name="xpool", bufs=cfg["xbufs"]) as xpool,
            tc.tile_pool(name="qtp", bufs=cfg["qtbufs"]) as qtp,
            tc.tile_pool(name="aop", bufs=2) as aop,
            tc.tile_pool(name="big", bufs=1) as big,
            tc.tile_pool(name="epool", bufs=cfg["ebufs"]) as epool,
            tc.tile_pool(name="spool", bufs=cfg["sbufs"]) as spool,
            tc.tile_pool(name="opool", bufs=2) as opool,
            tc.tile_pool(name="psq", bufs=cfg["psq"], space="PSUM") as psq,
            tc.tile_pool(name="psv", bufs=cfg["psv"], space="PSUM") as psv,
            tc.tile_pool(name="pso", bufs=cfg.get("pso", 1), space="PSUM") as pso,
        ):
            # ---- constants ----
            if tick:
                tick_sb = const.tile([128, 8], F32)
                nc.sync.dma_start(tick_sb, tick_ap)
            ones_plane = const.tile([128, 128], F32)
            nc.vector.memset(ones_plane, 1.0)
            ones65 = const.tile([65, 64], F32R)
            o65f = const.tile([65, 64], F32)
            nc.vector.memset(o65f, 1.0)
            nc.vector.tensor_copy(ones65, o65f)
            # head-pair broadcast selector [65, 128]: row 0 -> out rows
            # 0-63, row 64 -> out rows 64-127 (rows 1-63 zero; lg buffer
            # rows 1-63 are zeroed from DRAM so they contribute 0)
            selp = const.tile([65, 128], F32)
            nc.vector.memset(selp, 0.0)
            nc.vector.memset(selp[0:1, 0:64], 1.0)
            nc.vector.memset(selp[64:65, 64:128], 1.0)
            sel65 = const.tile([65, 128], F32R)
            nc.vector.tensor_copy(sel65, selp)
            lgbuf = const.tile([65, 2, SQT], F32R)
            nc.sync.dma_start(lgbuf, lgz.bitcast(F32R))

            if use_madd:
                ident_sb = const.tile([128, 128], BF16)
                nc.gpsimd.memset(ident_sb, 0.0)
                nc.gpsimd.affine_select(
                    out=ident_sb,
                    in_=ident_sb,
                    compare_op=mybir.AluOpType.not_equal,
                    fill=1.0,
                    base=0,
                    pattern=[[-1, 128]],
                    channel_multiplier=1,
                )
                madd_sb = const.tile([SKC, n_madd, SQT], BF16)
                nc.gpsimd.dma_start(madd_sb, madd.rearrange("n p s -> p n s"))
            wo_sb = const.tile([128, NMC, D], ao_dt)
            nc.sync.dma_start(wo_sb, wo.rearrange("(c p) m -> p c m", p=128))

            def emit_body():
                # ---- persistent tiles ----
                kT_sb = big.tile([128, NMC, S], qk_dt, tag="kT")
                v_sb = big.tile([128, NSKC, NHPC, DH + 1], pv_dt, tag="v")
                nc.vector.tensor_copy(
                    v_sb[:, :, :, DH : DH + 1],
                    ones_plane.rearrange("p (a b c) -> p a b c", a=NSKC, b=NHPC),
                )

                def load_xT(src, n):
                    xt = xpool.tile([128, NDC, SQT], in_dt, tag="xt")
                    nc.sync.dma_start(
                        xt,
                        src[:, n * SQT : (n + 1) * SQT].rearrange(
                            "(c p) s -> p c s", p=128
                        ),
                    )
                    return xt

                def load_w(wsrc):
                    wt = wbig.tile([128, NDC, HDPC], in_dt, tag="wt")
                    nc.sync.dma_start(
                        wt, wsrc.rearrange("(c p) m -> p c m", p=128)
                    )
                    return wt

                # ---- deferred work helpers ----
                ep_slot = [0]

                def epilogue_pair(pvA, pvB, hA, hB, aoT_t):
                    # 1/den via exp(-ln(den)): two lns on ACT ([1,512],
                    # cheap), one PE broadcast of both heads' ln-dens via
                    # sel65, one exp(scale=-1) -> per-query reciprocals
                    # for both heads. Avoids the DVE reciprocal (8
                    # cyc/elem = 3.4us serial, stalled the PE pipeline).
                    mc_ = hA // 2
                    slot = ep_slot[0]
                    ep_slot[0] ^= 1
                    lg = lgbuf[:, slot, :]
                    with nc.allow_low_precision("log softmax denom"):
                        nc.scalar.activation(
                            lg[0:1, :], pvA[64:65, :],
                            mybir.ActivationFunctionType.Ln,
                        )
                        nc.scalar.activation(
                            lg[64:65, :], pvB[64:65, :],
                            mybir.ActivationFunctionType.Ln,
                        )
                    bc = psq.tile([128, SQT], F32, tag="ps512")
                    nc.tensor.matmul(bc, sel65, lg)
                    bcast = spool.tile([128, SQT], F32, tag="bcast")
                    nc.scalar.activation(
                        bcast, bc, mybir.ActivationFunctionType.Exp, scale=-1.0
                    )
                    tmpA = spool.tile([64, SQT], ao_dt, tag="tmp")
                    tmpB = spool.tile([64, SQT], ao_dt, tag="tmp")
                    nc.vector.tensor_mul(tmpA, pvA[0:64, :], bcast[0:64, :])
                    nc.vector.tensor_mul(tmpB, pvB[0:64, :], bcast[64:128, :])
                    nc.sync.dma_start(aoT_t[0:64, mc_, :], tmpA)
                    nc.sync.dma_start(aoT_t[64:128, mc_, :], tmpB)

                def outproj_sc(aoT_prev, sc, flush=False):
                    si = sc % 4
                    for j in range(2):
                        # at flush time the attention psq pool is idle --
                        # rotate its 3 banks so the chains pipeline
                        if flush:
                            po = psq.tile([128, 512], F32, tag="ps512")
                        else:
                            po = pso.tile([128, 512], F32, tag="po")
                        for mc2 in range(NMC):
                            nc.tensor.matmul(
                                po,
                                aoT_prev[:, mc2, si * 128 : (si + 1) * 128],
                                wo_sb[:, mc2, j * 512 : (j + 1) * 512],
                                start=(mc2 == 0),
                                stop=(mc2 == NMC - 1),
                            )
                        o_sb = opool.tile([128, 512], F32, tag="o")
                        nc.vector.tensor_copy(o_sb, po)
                        nc.sync.dma_start(
                            out[sc * 128 : (sc + 1) * 128, j * 512 : (j + 1) * 512],
                            o_sb,
                        )

                # ---- projection chain helpers ----
                def kchain(t1, xt, m):
                    ps = psq.tile([128, SQT], F32, tag="ps512")
                    for kc in range(NDC):
                        nc.tensor.matmul(
                            ps,
                            wtk[:, kc, m * 128 : (m + 1) * 128],
                            xt[:, kc, :],
                            start=(kc == 0),
                            stop=(kc == NDC - 1),
                        )
                    nc.vector.tensor_copy(
                        kT_sb[:, m, t1 * SQT : (t1 + 1) * SQT], ps
                    )

                def vchain(t1, xt, si):
                    sc = t1 * 4 + si
                    ps = psq.tile([128, SQT], F32, tag="ps512")
                    for kc in range(NDC):
                        nc.tensor.matmul(
                            ps,
                            xt[:, kc, si * 128 : (si + 1) * 128],
                            wtv[:, kc, :],
                            start=(kc == 0),
                            stop=(kc == NDC - 1),
                        )
                    nc.vector.tensor_copy(
                        v_sb[:, sc, :, 0:DH],
                        ps.rearrange("p (h e) -> p h e", h=NHPC),
                    )

                def qchain(qT, xt, m):
                    ps = psq.tile([128, SQT], F32, tag="ps512")
                    for kc in range(NDC):
                        nc.tensor.matmul(
                            ps,
                            wtq[:, kc, m * 128 : (m + 1) * 128],
                            xt[:, kc, :],
                            start=(kc == 0),
                            stop=(kc == NDC - 1),
                        )
                    nc.vector.tensor_copy(qT[:, m, :], ps)

                def make_proj_fillers(t1):
                    # emit tile t1's K/V/Q proj chains lazily, 3 per
                    # head-pair boundary of tile t1-1's attention, so the
                    # PE chews projection work inside the ACT-paced
                    # attention instead of a serial ACT-idle proj phase
                    xtk1 = load_xT(xkT, t1)
                    xtv1 = load_xT(xvT, t1)
                    xtq1 = load_xT(xqT, t1)
                    qT1 = qtp.tile([128, NMC, SQT], qk_dt, tag="qT")
                    fillers = []
                    for m in range(NMC):
                        fillers.append(lambda m=m: kchain(t1, xtk1, m))
                    for si in range(4):
                        fillers.append(lambda si=si: vchain(t1, xtv1, si))
                    for m in range(NMC):
                        fillers.append(lambda m=m: qchain(qT1, xtq1, m))
                    return fillers, qT1

                # ---- per sq-tile: attention over chunks 0..4t+3
                # (causality: only K/V blocks <= t needed) with
                # outproj(t-1) and proj(t+1) interleaved at pair
                # boundaries ----
                # tile 0's minimal prefix runs up front: head-pair 0 of
                # tile 0 only needs kT m=0, all four v chunks, and qT
                # m=0 -- the remaining 6 chains become boundary fillers
                # so attention (and the ACT pacer) starts ~11us earlier.
                # Load order matters: the DMA queue is serial, so each
                # projection's inputs are queued right before its chains.
                prev_epi = None  # (pvA, pvB, hA, hB, aoT_t)
                aoT_prev = None
                wtk = load_w(wk)
                xtk = load_xT(xkT, 0)
                wtv = wtq = None
                kchain(0, xtk, 0)
                wtv = load_w(wv)
                xtv = load_xT(xvT, 0)
                for si in range(4):
                    vchain(0, xtv, si)
                wtq = load_w(wq)
                xtq = load_xT(xqT, 0)
                qT_next = qtp.tile([128, NMC, SQT], qk_dt, tag="qT")
                qchain(qT_next, xtq, 0)
                leftover = []
                for m in range(1, NMC):
                    leftover.append(
                        lambda m=m, x=xtk: kchain(0, x, m)
                    )
                    leftover.append(
                        lambda m=m, q=qT_next, x=xtq: qchain(q, x, m)
                    )

                for t in range(NSQT):
                    qT_t = qT_next
                    if t + 1 < NSQT:
                        nf, qT_next = make_proj_fillers(t + 1)
                    else:
                        nf, qT_next = [], None
                    fillers = leftover + nf
                    leftover = []
                    # ceil-first split over the 4 pair boundaries keeps
                    # tile-0's k/q chains ahead of the pairs needing them
                    nfil = len(fillers)
                    per = [nfil // 4 + (1 if i < nfil % 4 else 0)
                           for i in range(4)]
                    foff = [0]
                    for p_ in per:
                        foff.append(foff[-1] + p_)
                    aoT_t = aop.tile([128, NMC, SQT], ao_dt, tag="aoT")
                    for hp in range(NHPC // 2):
                        hA, hB = 2 * hp, 2 * hp + 1
                        qsA = qT_t[0:64, hp, :]
                        qsB = qT_t[64:128, hp, :]
                        pvA = psv.tile([65, SQT], F32, tag="pv")
                        pvB = psv.tile([65, SQT], F32, tag="pv")
                        pend = None
                        for ci, (c, kind, arg) in enumerate(chunks[t]):
                            qkA = psq.tile([128, SQT], F32, tag="ps512")
                            qkB = psq.tile([128, SQT], F32, tag="ps512")
                            last = kind != "madd"
                            nc.tensor.matmul(
                                qkA,
                                kT_sb[0:64, hp, c * SKC : (c + 1) * SKC],
                                qsA,
                                start=True,
                                stop=last,
                            )
                            nc.tensor.matmul(
                                qkB,
                                kT_sb[64:128, hp, c * SKC : (c + 1) * SKC],
                                qsB,
                                start=True,
                                stop=last,
                            )
                            if kind == "madd":
                                nc.tensor.matmul(
                                    qkA, ident_sb, madd_sb[:, arg, :],
                                    start=False, stop=True,
                                )
                                nc.tensor.matmul(
                                    qkB, ident_sb, madd_sb[:, arg, :],
                                    start=False, stop=True,
                                )
                            eA = epool.tile([SKC, SQT], pv_dt, tag="e")
                            eB = epool.tile([SKC, SQT], pv_dt, tag="e")
                            nc.scalar.activation(
                                eA, qkA, mybir.ActivationFunctionType.Exp
                            )
                            nc.scalar.activation(
                                eB, qkB, mybir.ActivationFunctionType.Exp
                            )
                            if kind == "affine":
                                # keep e[r, j] iff j - r - base >= 0 (i.e. sk <= sq)
                                for e_ in (eA, eB):
                                    nc.gpsimd.affine_select(
                                        out=e_,
                                        in_=e_,
                                        compare_op=mybir.AluOpType.is_ge,
                                        fill=0.0,
                                        base=-arg,
                                        pattern=[[1, SQT]],
                                        channel_multiplier=-1,
                                    )
                            if pend is not None:
                                pc, peA, peB, pci = pend
                                nc.tensor.matmul(
                                    pvA, v_sb[:, pc, hA, :], peA,
                                    start=(pci == 0), stop=False,
                                )
                                nc.tensor.matmul(
                                    pvB, v_sb[:, pc, hB, :], peB,
                                    start=(pci == 0), stop=False,
                                )
                            pend = (c, eA, eB, ci)
                            if ci == 1 and prev_epi is not None:
                                ppA, ppB, phA, phB, ao_ = prev_epi
                                epilogue_pair(ppA, ppB, phA, phB, ao_)
                                prev_epi = None
                        pc, peA, peB, pci = pend
                        nc.tensor.matmul(
                            pvA, v_sb[:, pc, hA, :], peA, start=(pci == 0), stop=True
                        )
                        nc.tensor.matmul(
                            pvB, v_sb[:, pc, hB, :], peB, start=(pci == 0), stop=True
                        )
                        prev_epi = (pvA, pvB, hA, hB, aoT_t)
                        if aoT_prev is not None:
                            outproj_sc(aoT_prev, (t - 1) * 4 + hp)
                        for _f in fillers[foff[hp] : foff[hp + 1]]:
                            _f()
                    aoT_prev = aoT_t

                # flush: last pair epilogue + last tile's output projection
                pvA, pvB, hA, hB, ao_ = prev_epi
                epilogue_pair(pvA, pvB, hA, hB, ao_)
                for si in range(4):
                    outproj_sc(aoT_prev, (NSQT - 1) * 4 + si, flush=True)

            for _rep in range(reps):
                emit_body()

    # patch scoped to finalize (where insert_act_table_loads consults
    # the table map) so the process-global state is restored after
    bacc.get_activation_tables = _patched_get_act_tables
    try:
        nc.finalize()
    finally:
        bacc.get_activation_tables = _orig_get_act_tables
    return nc


_PROG_CACHE = {}


def kernel(x_q, x_k, x_v, mask, Wq, Wk, Wv, Wo):
    global LAST_RESULTS
    x_q = np.asarray(x_q, dtype=np.float32)
    x_k = np.asarray(x_k, dtype=np.float32)
    x_v = np.asarray(x_v, dtype=np.float32)
    mask = np.asarray(mask).astype(bool)
    Wq = np.asarray(Wq, dtype=np.float32)
    Wk = np.asarray(Wk, dtype=np.float32)
    Wv = np.asarray(Wv, dtype=np.float32)
    Wo = np.asarray(Wo, dtype=np.float32)

    chunks, madd_arr, use_madd = _mask_layout(mask)
    key = (
        tuple(tuple(lst) for lst in chunks),
        madd_arr.shape[0],
        use_madd,
        tuple(sorted(CFG.items())),
    )
    if key not in _PROG_CACHE:
        _PROG_CACHE[key] = _build_program(
            chunks, madd_arr.shape[0], use_madd, CFG
        )
    nc = _PROG_CACHE[key]

    import ml_dtypes

    in_np = ml_dtypes.bfloat16 if CFG.get("in") == "bf16" else np.float32
    wq_s = np.ascontiguousarray(Wq * np.float32(SCALE))
    lgz_arr = np.zeros((65, 2, SQT), dtype=np.float32)
    xqTb = [np.ascontiguousarray(x_q[b].T).astype(in_np) for b in range(B)]
    xkTb = [np.ascontiguousarray(x_k[b].T).astype(in_np) for b in range(B)]
    xvTb = [np.ascontiguousarray(x_v[b].T).astype(in_np) for b in range(B)]
    in_maps = []
    for c in range(NCORES):
        b = c // 2
        hs = slice((c % 2) * HDPC, (c % 2 + 1) * HDPC)
        m = {
            "xqT": xqTb[b],
            "xkT": xkTb[b],
            "xvT": xvTb[b],
            "wq": np.ascontiguousarray(wq_s[:, hs]).astype(in_np),
            "wk": np.ascontiguousarray(Wk[:, hs]).astype(in_np),
            "wv": np.ascontiguousarray(Wv[:, hs]).astype(in_np),
            "wo": np.ascontiguousarray(Wo[hs, :]).astype(in_np),
            "lgz": lgz_arr,
        }
        if use_madd:
            m["madd"] = madd_arr
        in_maps.append(m)

    res = run_bass_kernel_spmd(nc, in_maps, core_ids=list(range(NCORES)))
    LAST_RESULTS = res
    out = np.empty((B, S, D), dtype=np.float32)
    for b in range(B):
        out[b] = res.results[2 * b]["out"] + res.results[2 * b + 1]["out"]
    return out

